# revision 2
# baseline (speedup 1.0000x reference)
"""Trainium2 Bass kernel for NT-Xent contrastive loss (BATCH=4096, DIM=512, TEMP=0.5).

Strategy (data-parallel over rows of the 2B x 2B similarity matrix):
  - Host: E = concat(emb_i, emb_j) [8192, 512] f32 -> ET = E.T as bf16 [512, 8192].
    Each core receives the full ET (replicated) plus its own 1024-column block and
    the partner block (rows shifted by +-B, for the positive pairs).
  - Device (per core, SPMD, no collectives):
      * sumsq of every row via PE Gram-diagonals + DVE diag-extract -> r = 1/||e||
      * normalize the rhs copy column-wise: z_j = e_j * r_j  (r broadcast via
        DRAM-bounce + step-0 DMA)
      * S' = e_block^T @ Z via PE (bf16, fp32 accum), in [128, 1024] PSUM chunks
      * ACT: exp(S' * r_row/TEMP) with fused row-sum accumulation -> denominators
      * positives via PE diag of e_block^T @ e_partner, scaled by r_own*r_partner/TEMP
      * per-core partial: sum_rows(log(den - e^{1/TEMP}) - pos/TEMP) -> [1,1] f32
  - Host: loss = sum(partials) / (2B).
"""

import math

import ml_dtypes
import numpy as np

BATCH = 4096
DIM = 512
TEMP = 0.5
B2 = 2 * BATCH              # 8192 rows/cols of the similarity matrix
NCORES = 8
RPC = B2 // NCORES          # 1024 rows per core
KT = DIM // 128             # 4 contraction chunks
CG = 8                      # column groups
CGW = B2 // CG              # 1024 columns per group
T8 = RPC // 128             # 8 own row-tiles
NBF = CGW // 512            # 512-wide matmuls per group
EXP_DIAG = math.exp(1.0 / TEMP)

_CACHE = {}


def _build():
    import concourse.bass as bass
    import concourse.bacc as bacc
    import concourse.mybir as mybir
    import concourse.tile as tile

    f32 = mybir.dt.float32
    bf16 = mybir.dt.bfloat16
    AF = mybir.ActivationFunctionType
    ALU = mybir.AluOpType
    X = mybir.AxisListType.X

    nc = bacc.Bacc("TRN2", target_bir_lowering=False, debug=False,
                   num_devices=NCORES)

    et_d = nc.dram_tensor("et", [DIM, B2], bf16, kind="ExternalInput").ap()
    etb_d = nc.dram_tensor("etb", [DIM, RPC], bf16, kind="ExternalInput").ap()
    etp_d = nc.dram_tensor("etp", [DIM, RPC], bf16, kind="ExternalInput").ap()
    iden_d = nc.dram_tensor("iden", [128, 128], f32, kind="ExternalInput").ap()
    out_d = nc.dram_tensor("out", [1, 1], f32, kind="ExternalOutput").ap()
    rflat = [nc.dram_tensor(f"rflat{c}", [CGW], bf16) for c in range(CG)]

    with tile.TileContext(nc) as tc:
        with (
            tc.tile_pool(name="persist", bufs=1) as P,
            tc.tile_pool(name="scratch", bufs=2) as S,
            tc.tile_pool(name="psum", bufs=3, space="PSUM") as PS,
        ):
            et = [[P.tile([128, CGW], bf16, name=f"et_{k}_{c}")
                   for c in range(CG)] for k in range(KT)]
            etn = [[P.tile([128, CGW], bf16, name=f"etn_{k}_{c}")
                    for c in range(CG)] for k in range(KT)]
            rbc = [P.tile([128, CGW], bf16, name=f"rbc_{c}") for c in range(CG)]
            etb = [P.tile([128, RPC], bf16, name=f"etb_{k}") for k in range(KT)]
            etp = [P.tile([128, RPC], bf16, name=f"etp_{k}") for k in range(KT)]
            iden = P.tile([128, 128], f32, name="iden")
            ss64 = P.tile([128, 64], f32, name="ss64")
            ssb = P.tile([128, T8], f32, name="ssb")
            ssp = P.tile([128, T8], f32, name="ssp")
            rawpos = P.tile([128, T8], f32, name="rawpos")
            rsums = P.tile([128, T8 * CG], f32, name="rsums")
            sc8 = P.tile([128, T8], f32, name="sc8")
            pos8 = P.tile([128, T8], f32, name="pos8")
            ones = P.tile([128, 1], f32, name="ones")

            nc.sync.dma_start(iden[:], iden_d[:])
            nc.vector.memset(ones[:], 1.0)
            for k in range(KT):
                nc.sync.dma_start(etb[k][:], etb_d[k * 128:(k + 1) * 128, :])
                nc.sync.dma_start(etp[k][:], etp_d[k * 128:(k + 1) * 128, :])
            for c in range(CG):
                for k in range(KT):
                    nc.sync.dma_start(
                        et[k][c][:],
                        et_d[k * 128:(k + 1) * 128, c * CGW:(c + 1) * CGW])

            def diag_accum(lhs, rhs, col, dst, dcol):
                """dst[:, dcol] = diag(sum_k lhs[k][:, col:+128].T @ rhs[k][:, col:+128])"""
                ps = PS.tile([128, 128], f32, tag="diag", bufs=2, name="psd")
                for k in range(KT):
                    nc.tensor.matmul(ps[:], lhs[k][:, col:col + 128],
                                     rhs[k][:, col:col + 128],
                                     start=(k == 0), stop=(k == KT - 1))
                sco = S.tile([128, 128], f32, tag="stt", name="sco")
                nc.vector.scalar_tensor_tensor(
                    sco[:], ps[:], 1.0, iden[:], ALU.mult, ALU.mult,
                    accum_out=dst[:, dcol:dcol + 1])

            # own/partner norms + raw positive dots
            for t in range(T8):
                diag_accum(etb, etb, t * 128, ssb, t)
                diag_accum(etp, etp, t * 128, ssp, t)
                diag_accum(etb, etp, t * 128, rawpos, t)

            nb8 = S.tile([128, T8], f32, tag="nrm", name="nb8")
            nc.scalar.activation(nb8[:], ssb[:], AF.Sqrt)
            rb8 = P.tile([128, T8], f32, name="rb8")
            nc.vector.reciprocal(rb8[:], nb8[:])
            np8 = S.tile([128, T8], f32, tag="nrm", name="np8")
            nc.scalar.activation(np8[:], ssp[:], AF.Sqrt)
            rp8 = P.tile([128, T8], f32, name="rp8")
            nc.vector.reciprocal(rp8[:], np8[:])
            nc.vector.tensor_scalar_mul(sc8[:], rb8[:], 1.0 / TEMP)
            pt0 = P.tile([128, T8], f32, name="pt0")
            nc.vector.tensor_mul(pt0[:], rawpos[:], rb8[:])
            pt1 = P.tile([128, T8], f32, name="pt1")
            nc.vector.tensor_mul(pt1[:], pt0[:], rp8[:])
            nc.vector.tensor_scalar_mul(pos8[:], pt1[:], 1.0 / TEMP)

            # full-row norms per column group; broadcast; normalize rhs
            for c in range(CG):
                for tt in range(T8):
                    diag_accum([et[k][c] for k in range(KT)],
                               [et[k][c] for k in range(KT)],
                               tt * 128, ss64, c * 8 + tt)
                nsq = S.tile([128, T8], f32, tag="nsq", name="nsq")
                nc.scalar.activation(nsq[:], ss64[:, c * 8:(c + 1) * 8], AF.Sqrt)
                rcp = S.tile([128, T8], f32, tag="rcp", name="rcp")
                nc.vector.reciprocal(rcp[:], nsq[:])
                rcb = S.tile([128, T8], bf16, tag="rcb", name="rcb")
                nc.vector.tensor_copy(rcb[:], rcp[:])
                # rbc[q, t*128+p] = rcb[p, t] for every partition q
                nc.sync.dma_start(bass.AP(rflat[c], 0, [[1, 128], [128, T8]]),
                                  rcb[:])
                nc.sync.dma_start(rbc[c][:],
                                  bass.AP(rflat[c], 0, [[0, 128], [1, CGW]]))
                for k in range(KT):
                    nc.vector.tensor_mul(etn[k][c][:], et[k][c][:], rbc[c][:])

            # main loop: row-block of similarity + fused exp/row-sum
            for c in range(CG):
                for t in range(T8):
                    ps = PS.tile([128, CGW], f32, tag="mm", bufs=3, name="psmm")
                    for k in range(KT):
                        for n in range(NBF):
                            nc.tensor.matmul(
                                ps[:, n * 512:(n + 1) * 512],
                                etb[k][:, t * 128:(t + 1) * 128],
                                etn[k][c][:, n * 512:(n + 1) * 512],
                                start=(k == 0), stop=(k == KT - 1))
                    sce = S.tile([128, CGW], bf16, tag="expout", name="sce")
                    col = t * CG + c
                    nc.scalar.activation(sce[:], ps[:], AF.Exp,
                                         scale=sc8[:, t:t + 1],
                                         accum_out=rsums[:, col:col + 1])

            # finalize: den = rowsum - e^{1/T}; partial = sum(log(den) - pos)
            den8 = P.tile([128, T8], f32, name="den8")
            nc.vector.tensor_reduce(
                den8[:], rsums[:].rearrange("p (t c) -> p t c", c=CG),
                X, ALU.add)
            den8b = P.tile([128, T8], f32, name="den8b")
            nc.vector.tensor_scalar_add(den8b[:], den8[:], -EXP_DIAG)
            logd = S.tile([128, T8], f32, tag="logd", name="logd")
            tlog = P.tile([128, 1], f32, name="tlog")
            nc.scalar.activation(logd[:], den8b[:], AF.Ln, accum_out=tlog[:])
            tpos = P.tile([128, 1], f32, name="tpos")
            nc.vector.tensor_reduce(tpos[:], pos8[:], X, ALU.add)
            lv = P.tile([128, 1], f32, name="lv")
            nc.vector.tensor_sub(lv[:], tlog[:], tpos[:])
            psf = PS.tile([1, 1], f32, tag="diag", bufs=2, name="psf")
            nc.tensor.matmul(psf[:], lv[:], ones[:], start=True, stop=True)
            ob = P.tile([1, 1], f32, name="ob")
            nc.vector.tensor_copy(ob[:], psf[:])
            nc.sync.dma_start(out_d[:], ob[:])

    nc.compile()
    return nc


def _get_nc():
    if "nc" not in _CACHE:
        _CACHE["nc"] = _build()
    return _CACHE["nc"]


def _in_maps(emb_i, emb_j):
    bf = ml_dtypes.bfloat16
    E = np.concatenate([np.asarray(emb_i, dtype=np.float32),
                        np.asarray(emb_j, dtype=np.float32)], axis=0)
    ET = np.ascontiguousarray(E.T).astype(bf)       # [512, 8192]
    iden = np.eye(128, dtype=np.float32)
    maps = []
    for k in range(NCORES):
        s = k * RPC
        p = (s + BATCH) % B2
        maps.append({
            "et": ET,
            "etb": np.ascontiguousarray(ET[:, s:s + RPC]),
            "etp": np.ascontiguousarray(ET[:, p:p + RPC]),
            "iden": iden,
        })
    return maps


def _run(emb_i, emb_j, trace=False):
    from concourse.bass_utils import run_bass_kernel_spmd
    nc = _get_nc()
    res = run_bass_kernel_spmd(nc, _in_maps(emb_i, emb_j),
                               list(range(NCORES)), trace=trace)
    total = sum(float(res.results[i]["out"][0, 0]) for i in range(NCORES))
    loss = np.float32(total / B2)
    return loss, res


def kernel(emb_i, emb_j):
    return _run(emb_i, emb_j, trace=False)[0]
